# revision 7
# baseline (speedup 1.0000x reference)
"""DistMult decoder kernel for 8 Trainium2 NeuronCores.

Computes out = (input1 * weight[type_index]) @ input2.T + bias with
input1 [8192, 512], input2 [8192, 512] in fp32, out [8192, 8192].

Sharding: rows of input1 (and thus rows of the output) are split across
the 8 cores; input2 / weight / bias are replicated. No communication.

Per-core device program (M = 1024 rows):
  - lhsT  [512, 1024]  = w_r-scaled shard of input1, transposed + cast
    to fp16 on host (K-major)
  - rhs   [512, 8192]  = input2 transposed + cast to fp16 on host
  - fp16 operands run the PE at 1 col/cycle (4x fp32) with fp32 PSUM
    accumulation
  - output stored as fp16 (max |out| ~ 128, fp16 quantization adds
    ~5e-4 max-rel error; host upcasts to fp32) which halves the store
    traffic: 17 MB instead of 34 MB per core, taking DMA off the
    critical path (PE floor ~109 us, DMA now ~75 us)
  - loop structure: 16 halfgroups of 512 output columns; within one
    halfgroup, k-outer/m-inner over all 8 PSUM banks, so the first
    matmul only needs the first k-slice of rhs (128 KB) and of lhsT
    (256 KB) instead of the whole first column block
  - PSUM evacuation + bias add split between DVE (even m) and ACT
    (odd m), output stores alternate between the two HWDGE rings
"""

import os

import numpy as np

import concourse.bacc as bacc
import concourse.mybir as mybir
from concourse.bass_utils import run_bass_kernel_spmd
from concourse.tile import TileContext

N_CORES = 8
N1, N2, D = 8192, 8192, 512
M = N1 // N_CORES  # rows per core
P = 128            # partitions
KT = D // P        # 4 k-tiles
MT = M // P        # 8 m-tiles
HGW = 512          # halfgroup width = one psum bank of fp32
HG = N2 // HGW     # 16 halfgroups

# test.py hooks: set TRACE=True before calling kernel() to profile; the
# BassKernelResults of the last run lands in LAST_RESULTS.
TRACE = os.environ.get("BASS_KERNEL_TRACE", "0") == "1"
LAST_RESULTS = None

_cached_nc = None


def _build():
    nc = bacc.Bacc(
        "TRN2", target_bir_lowering=False, debug=False, enable_asserts=False, num_devices=N_CORES
    )
    f32 = mybir.dt.float32
    f16 = mybir.dt.float16
    lhsT = nc.dram_tensor("lhsT", [D, M], f16, kind="ExternalInput")
    rhs = nc.dram_tensor("rhs", [D, N2], f16, kind="ExternalInput")
    biasv = nc.dram_tensor("biasv", [P, 1], f32, kind="ExternalInput")
    out = nc.dram_tensor("out", [M, N2], f16, kind="ExternalOutput")

    # K-major DRAM views split into [P, KT, cols] for single-DMA loads.
    lhsT_r = lhsT[:, :].rearrange("(kt p) m -> p kt m", p=P)
    rhs_r = rhs[:, :].rearrange("(kt p) n -> p kt n", p=P)

    with TileContext(nc) as tc:
        with (
            tc.tile_pool(name="const", bufs=1) as constp,
            tc.tile_pool(name="lhs", bufs=1) as lhsp,
            tc.tile_pool(name="rhsp", bufs=4) as rhsp,
            tc.tile_pool(name="outp", bufs=16) as outp,
            tc.tile_pool(name="psum", bufs=8, space="PSUM") as psump,
        ):
            # Head loads. The binding constraint is the first matmul
            # (k=0, m=0): it needs only rhs k-slice 0 of halfgroup 0 and
            # lhsT k-slice 0, so those go first on the Sync HWDGE ring
            # (fastest first-byte). Each dma_start costs ~0.7 us of
            # issue time on its engine, so the rest is batched coarsely:
            # Sync carries the k=1..3 remainder of halfgroup 0, GpSimd
            # (SWDGE) carries the lhsT remainder and the halfgroup 1/2
            # prefetch. Scalar is blocked by the framework's
            # ACT_TABLE_LOAD until ~8.3 us, so it only gets the bias.
            lt = lhsp.tile([P, KT, M], f16, tag="lhs")
            rts = {}

            # The two DMAs gating the first matmul (rhs k-slice 0 and
            # lhsT k-slice 0) go out in parallel on different rings so
            # their ~2 us completion-receipt latencies overlap.
            rt0 = rhsp.tile([P, KT, HGW], f16, tag="rhs")
            rts[0] = rt0
            nc.sync.dma_start(out=rt0[:, 0, :], in_=rhs_r[:, 0, 0:HGW])
            nc.gpsimd.dma_start(out=lt[:, 0, :], in_=lhsT_r[:, 0, :])
            nc.sync.dma_start(out=rt0[:, 1:KT, :], in_=rhs_r[:, 1:KT, 0:HGW])
            nc.gpsimd.dma_start(out=lt[:, 1:KT, :], in_=lhsT_r[:, 1:KT, :])
            bias_t = constp.tile([P, 1], f32, tag="bias")
            nc.scalar.dma_start(out=bias_t[:], in_=biasv[:, :])

            def load_rhs(g, eng):
                rt = rhsp.tile([P, KT, HGW], f16, tag="rhs")
                eng.dma_start(out=rt[:], in_=rhs_r[:, :, g * HGW : (g + 1) * HGW])
                rts[g] = rt

            load_rhs(1, nc.gpsimd)
            load_rhs(2, nc.gpsimd)

            # Warm up the PE's HAM clock gate while the head loads are
            # in flight: ~2 us of small matmuls on zeroed SBUF so the PE
            # is busy from ~7.4 us (right after the framework preamble)
            # and flips to 2.4 GHz ~3.4 us later. Kept short so the real
            # matmuls don't queue behind it once their data lands.
            warm_w = constp.tile([P, P], f16, tag="warmw")
            warm_r = constp.tile([P, P], f16, tag="warmr")
            nc.vector.memset(warm_w[:], 0.0)
            nc.vector.memset(warm_r[:], 0.0)
            wps = psump.tile([P, HGW], f32, tag="ps", name="wps")
            NWARM = 26
            for i in range(NWARM):
                nc.tensor.matmul(
                    wps[:, 0:P], warm_w[:], warm_r[:],
                    start=(i == 0), stop=(i == NWARM - 1),
                )

            for g in range(HG):
                rt = rts.pop(g)
                # Keep two halfgroups of rhs lookahead (g+1 and g+2 are
                # in flight when g starts).
                if g + 3 <= HG - 1:
                    load_rhs(g + 3, nc.gpsimd)
                # k-outer over all 8 psum banks: each rhs k-slice is
                # streamed through the PE for all 8 m-tiles before the
                # next k-slice is needed, so the head only waits on the
                # first 128 KB chunk, and each bank's accumulation
                # group finishes at k=3 with 7 matmuls of slack for the
                # evacuation engines.
                pss = [
                    psump.tile([P, HGW], f32, tag="ps", name=f"ps_{g}_{m}")
                    for m in range(MT)
                ]
                for k in range(KT):
                    for m in range(MT):
                        nc.tensor.matmul(
                            pss[m][:], lt[:, k, m * P : (m + 1) * P],
                            rt[:, k, :],
                            start=(k == 0), stop=(k == KT - 1),
                        )
                # Bias-add + fp32->fp16 cast on the way out of PSUM.
                # Evacuation is split across DVE and ACT; each engine's
                # per-tile cost (~0.7 us, PSUM-read errata) is slower
                # than the 216 ns matmul cadence, so the last banks of
                # the halfgroup (6, 7) get one engine each to minimize
                # the latency before the next halfgroup reuses them.
                # Stores stay off the Scalar engine (it runs ACT) --
                # Sync takes most, GpSimd (otherwise idle between rhs
                # prefetches) takes the rest. Emission order puts m=7
                # (evacuated earliest of the late banks) before m=6.
                ots = {}
                for m in range(MT):
                    ot = outp.tile([P, HGW], f16, tag="ot", name=f"ot_{g}_{m}")
                    if m in (0, 2, 4, 7):
                        nc.vector.tensor_scalar_add(ot[:], pss[m][:], bias_t[:, 0:1])
                    else:
                        nc.scalar.activation(
                            ot[:], pss[m][:],
                            mybir.ActivationFunctionType.Identity,
                            bias=bias_t[:, 0:1],
                        )
                    ots[m] = ot
                for m in (0, 2, 4, 7, 6, 1, 3, 5):
                    st = nc.sync if m in (0, 2, 4, 7, 6) else nc.gpsimd
                    st.dma_start(
                        out=out[m * P : (m + 1) * P, g * HGW : (g + 1) * HGW],
                        in_=ots[m][:],
                    )
    nc.compile()
    return nc


def kernel(input1, input2, weight, bias, type_index):
    global _cached_nc, LAST_RESULTS

    input1 = np.asarray(input1, dtype=np.float32)
    input2 = np.asarray(input2, dtype=np.float32)
    weight = np.asarray(weight, dtype=np.float32)
    bias = np.asarray(bias, dtype=np.float32).reshape(-1)
    w_r = weight[int(type_index)]  # [D]

    # Host-side prep: fold the w_r row-scale into input1, lay both GEMM
    # operands out K-major, cast to fp16 (device accumulates in fp32).
    scaled = input1 * w_r[None, :]  # [N1, D]
    rhsT = np.ascontiguousarray(input2.T.astype(np.float16))  # [D, N2]
    bias_vec = np.full((P, 1), float(bias[0]), dtype=np.float32)

    in_maps = []
    for c in range(N_CORES):
        shard = scaled[c * M : (c + 1) * M]  # [M, D]
        in_maps.append(
            {
                "lhsT": np.ascontiguousarray(shard.T.astype(np.float16)),
                "rhs": rhsT,
                "biasv": bias_vec,
            }
        )

    if _cached_nc is None:
        _cached_nc = _build()

    res = run_bass_kernel_spmd(
        _cached_nc, in_maps, core_ids=list(range(N_CORES)), trace=TRACE
    )
    LAST_RESULTS = res
    out = np.concatenate([res.results[c]["out"] for c in range(N_CORES)], axis=0)
    return out.astype(np.float32)
